# revision 6
# baseline (speedup 1.0000x reference)
# BitLinear 1.58 (ternary-weight linear with int8 activation quantization)
# on 8 Trainium2 NeuronCores via Bass/Tile.
#
# Reference computation (fp32):
#   w_scale = max(mean(|W|), 1e-5)           (global over the full weight)
#   W_q     = clip(round(W / w_scale), -1, 1)          (ternary)
#   gamma   = max(max(|x|), 1e-5)            (global over the full activation)
#   x_q     = clip(round(x * 128/gamma), -128, 127)
#   out     = (x_q @ W_q^T) * (gamma*w_scale/128) + bias
#
# Sharding: data-parallel over the 8192 tokens (1024 tokens/core), weight
# replicated. The global scales need cross-core reductions: each core
# computes a local absmax(x_shard) and a partial sum(|W|) over a distinct
# 1/8 slice of W, then one 16-byte AllGather shares both; each core then
# combines them locally (max over the 8 maxes, add over the 8 partial sums).
#
# The matmul contracts over in_features, which must live on the partition
# axis for both operands, so the host pre-transposes x and W once (layout
# prep, outside the device kernel). Quantized operands are fed to the PE in
# bf16 - exact here, because x_q in [-128,127] and W_q in {-1,0,1} are
# integers representable exactly in bf16, and PSUM accumulates in fp32
# (sums bounded by 4096*128 = 2^19 < 2^24, so accumulation is exact).
#
# Rounding: round-half-to-even (= jnp.round) done exactly in fp32 via the
# magic-constant trick (v + 1.5*2^23) - 1.5*2^23, fused into tensor_scalar
# ops. clip-then-round == round-then-clip at these bounds.

import numpy as np
from contextlib import ExitStack

import concourse.bass as bass
import concourse.tile as tile
from concourse import bacc, mybir
from concourse import bass_utils

N_CORES = 8
IN_F = 4096
OUT_F = 4096
TOKENS = 8192  # 4 * 2048
TPC = TOKENS // N_CORES  # tokens per core = 1024
OSL = OUT_F // N_CORES  # per-core weight-stats slice = 512 out_features

KT = IN_F // 128  # 32 k-tiles
CT = OUT_F // 512  # 8 of-columns
TT = TPC // 128  # 8 token-tiles

MAGIC = 12582912.0  # 1.5 * 2**23: (v + MAGIC) - MAGIC == round-half-even(v)
EPS = 1e-5
F32 = mybir.dt.float32
BF16 = mybir.dt.bfloat16

_cache = {}


def _build(dbg=False):
    nc = bacc.Bacc("TRN2", target_bir_lowering=False, debug=False, num_devices=N_CORES)
    xT = nc.dram_tensor("xT", [IN_F, TPC], F32, kind="ExternalInput").ap()
    wT = nc.dram_tensor("wT", [IN_F, OUT_F], F32, kind="ExternalInput").ap()
    wS = nc.dram_tensor("wS", [IN_F, OSL], F32, kind="ExternalInput").ap()
    bias = nc.dram_tensor("bias", [OUT_F], F32, kind="ExternalInput").ap()
    out = nc.dram_tensor("out", [TPC, OUT_F], F32, kind="ExternalOutput").ap()
    if dbg:
        dbg_t = nc.dram_tensor("dbg", [16], F32, kind="ExternalOutput").ap()

    with tile.TileContext(nc) as tc, ExitStack() as ctx:
        ep = ctx.enter_context
        singles = ep(tc.tile_pool(name="singles", bufs=1))
        xin_pool = ep(tc.tile_pool(name="xin", bufs=3))
        xtmp_pool = ep(tc.tile_pool(name="xtmp", bufs=3))
        xq_pool = ep(tc.tile_pool(name="xq", bufs=KT))
        win_pool = ep(tc.tile_pool(name="win", bufs=4))
        wtmp_pool = ep(tc.tile_pool(name="wtmp", bufs=4))
        wq_pool = ep(tc.tile_pool(name="wq", bufs=4))
        ost_pool = ep(tc.tile_pool(name="ost", bufs=4))
        psum_pool = ep(tc.tile_pool(name="psum", bufs=8, space="PSUM"))
        dram = ep(tc.tile_pool(name="dram", bufs=1, space="DRAM"))

        ones_col = singles.tile([128, 1], F32)  # for partition-sum matmul
        nc.vector.memset(ones_col[:], 1.0)
        ones_row = singles.tile([1, 128], F32)  # for partition-broadcast matmul
        nc.vector.memset(ones_row[:], 1.0)

        # ---- bias replicated across partitions (via K=1 matmul broadcast) ----
        bias_sb = singles.tile([1, OUT_F], F32)
        nc.sync.dma_start(bias_sb[:], bias[:])
        bias_rep = singles.tile([128, OUT_F], F32)
        for n in range(CT):
            bp = psum_pool.tile([128, 512], F32, tag="ps")
            nc.tensor.matmul(
                bp[:], ones_row[:], bias_sb[0:1, n * 512 : (n + 1) * 512],
                start=True, stop=True,
            )
            nc.scalar.copy(bias_rep[:, n * 512 : (n + 1) * 512], bp[:])

        # ---- stats: local absmax over x shard ----
        xm = singles.tile([128, KT], F32)
        for k in range(KT):
            xin = xin_pool.tile([128, TPC], F32)
            nc.sync.dma_start(xin[:], xT[k * 128 : (k + 1) * 128, :])
            nc.vector.tensor_reduce(
                xm[:, k : k + 1], xin[:], axis=mybir.AxisListType.X,
                op=mybir.AluOpType.max, apply_absolute_value=True,
            )
        xmax = singles.tile([128, 1], F32)
        nc.vector.tensor_reduce(
            xmax[:], xm[:], axis=mybir.AxisListType.X, op=mybir.AluOpType.max
        )
        xmaxT = singles.tile([1, 128], F32)
        nc.sync.dma_start(xmaxT[:], xmax[:])  # cross-partition reshape
        gx = singles.tile([1, 1], F32)
        nc.vector.tensor_reduce(
            gx[:], xmaxT[:], axis=mybir.AxisListType.X, op=mybir.AluOpType.max
        )

        # ---- stats: partial sum(|W|) over this core's slice ----
        wsum_ps = psum_pool.tile([128, OSL], F32, tag="ps")
        for k in range(KT):
            wsin = win_pool.tile([128, OSL], F32, tag="win")
            nc.sync.dma_start(wsin[:], wS[k * 128 : (k + 1) * 128, :])
            wabs = wtmp_pool.tile([128, OSL], F32, tag="wtmp")
            nc.scalar.activation(wabs[:], wsin[:], mybir.ActivationFunctionType.Abs)
            nc.tensor.matmul(
                wsum_ps[0:1, :], ones_col[:], wabs[:],
                start=(k == 0), stop=(k == KT - 1),
            )
        wsum = singles.tile([1, 1], F32)
        nc.vector.tensor_reduce(
            wsum[:], wsum_ps[0:1, :], axis=mybir.AxisListType.X,
            op=mybir.AluOpType.add,
        )

        # ---- share the two partial stats: one 8-byte-per-core AllGather ----
        cc_sb = singles.tile([1, 2], F32)
        nc.vector.tensor_copy(cc_sb[0:1, 0:1], gx[:])
        nc.vector.tensor_copy(cc_sb[0:1, 1:2], wsum[:])
        cc_in = dram.tile([2], F32)
        cc_out = dram.tile([2 * N_CORES], F32)
        nc.sync.dma_start(cc_in[:], cc_sb[:])
        nc.gpsimd.collective_compute(
            "AllGather", mybir.AluOpType.bypass,
            replica_groups=[list(range(N_CORES))],
            ins=[cc_in.opt()], outs=[cc_out.opt()],
        )
        g16 = singles.tile([1, 2 * N_CORES], F32)
        nc.sync.dma_start(g16[:], cc_out[:])
        # strided views: even slots = per-core absmax, odd = per-core |W| sums
        g3 = g16[:].rearrange("p (r two) -> p two r", two=2)
        gmax = singles.tile([1, 1], F32)
        nc.vector.tensor_reduce(
            gmax[:], g3[0:1, 0:1, :], axis=mybir.AxisListType.X,
            op=mybir.AluOpType.max,
        )
        gsum = singles.tile([1, 1], F32)
        nc.vector.tensor_reduce(
            gsum[:], g3[0:1, 1:2, :], axis=mybir.AxisListType.X,
            op=mybir.AluOpType.add,
        )

        # ---- derive the scalars exactly as the reference does ----
        # gamma = max(absmax, eps); w_scale = max(sum/2^24, eps)
        gamma = singles.tile([1, 1], F32)
        nc.vector.tensor_scalar(gamma[:], gmax[:], EPS, None, mybir.AluOpType.max)
        wscale = singles.tile([1, 1], F32)
        nc.vector.tensor_scalar(
            wscale[:], gsum[:], 1.0 / (OUT_F * IN_F), EPS,
            mybir.AluOpType.mult, mybir.AluOpType.max,
        )

        def newton_recip(name, src):
            # correctly-rounded-ish 1/src: HW reciprocal + one Newton step
            r0 = singles.tile([1, 1], F32, tag=f"{name}r0")
            nc.vector.reciprocal(r0[:], src[:])
            t = singles.tile([1, 1], F32, tag=f"{name}t")
            nc.vector.tensor_tensor(t[:], src[:], r0[:], op=mybir.AluOpType.mult)
            u = singles.tile([1, 1], F32, tag=f"{name}u")
            nc.vector.tensor_scalar(
                u[:], t[:], -1.0, 2.0, mybir.AluOpType.mult, mybir.AluOpType.add
            )
            r1 = singles.tile([1, 1], F32, tag=f"{name}r1")
            nc.vector.tensor_tensor(r1[:], r0[:], u[:], op=mybir.AluOpType.mult)
            return r1

        rg = newton_recip("rg", gamma)   # 1/gamma
        rw = newton_recip("rw", wscale)  # 1/w_scale

        # pack [s_x, r_w, s_out] then broadcast to all 128 partitions
        pack3 = singles.tile([1, 3], F32)
        nc.vector.tensor_scalar(
            pack3[0:1, 0:1], rg[:], 128.0, None, mybir.AluOpType.mult
        )
        nc.vector.tensor_copy(pack3[0:1, 1:2], rw[:])
        gws = singles.tile([1, 1], F32)
        nc.vector.tensor_tensor(gws[:], gamma[:], wscale[:], op=mybir.AluOpType.mult)
        nc.vector.tensor_scalar(
            pack3[0:1, 2:3], gws[:], 2.0 ** -7, None, mybir.AluOpType.mult
        )
        b3 = singles.tile([128, 3], F32)
        nc.gpsimd.partition_broadcast(b3[:], pack3[:])
        s_x = b3[:, 0:1]
        r_w = b3[:, 1:2]
        s_o = b3[:, 2:3]

        if dbg:
            dsb = singles.tile([1, 16], F32)
            nc.vector.memset(dsb[:], 0.0)
            nc.vector.tensor_copy(dsb[0:1, 0:1], gamma[:])
            nc.vector.tensor_copy(dsb[0:1, 1:2], wscale[:])
            nc.vector.tensor_copy(dsb[0:1, 2:5], pack3[:])
            nc.vector.tensor_copy(dsb[0:1, 5:8], b3[96:97, :])
            nc.vector.tensor_copy(dsb[0:1, 8:9], gmax[:])
            nc.vector.tensor_copy(dsb[0:1, 9:10], gsum[:])
            nc.vector.tensor_copy(dsb[0:1, 10:11], gx[:])
            nc.vector.tensor_copy(dsb[0:1, 11:12], wsum[:])
            nc.sync.dma_start(dbg_t[:], dsb[:])

        # ---- quantize x shard to bf16 int values, kept resident in SBUF ----
        xq = []
        for k in range(KT):
            xin = xin_pool.tile([128, TPC], F32)
            nc.sync.dma_start(xin[:], xT[k * 128 : (k + 1) * 128, :])
            xs = xtmp_pool.tile([128, TPC], F32)
            nc.scalar.activation(
                xs[:], xin[:], mybir.ActivationFunctionType.Copy, scale=s_x
            )
            nc.vector.tensor_scalar(
                xs[:], xs[:], 127.0, -128.0, mybir.AluOpType.min, mybir.AluOpType.max
            )
            xq_k = xq_pool.tile([128, TPC], BF16)
            nc.vector.tensor_scalar(
                xq_k[:], xs[:], MAGIC, MAGIC, mybir.AluOpType.add,
                mybir.AluOpType.subtract,
            )
            xq.append(xq_k)

        # ---- main loop: stream W, ternarize, matmul, fused evict ----
        for c in range(CT):
            of = c * 512
            psums = [
                psum_pool.tile([128, 512], F32, tag="ps", name=f"psum_c{c}_t{t}")
                for t in range(TT)
            ]
            for k in range(KT):
                win = win_pool.tile([128, 512], F32, tag="win")
                nc.sync.dma_start(
                    win[:], wT[k * 128 : (k + 1) * 128, of : of + 512]
                )
                ws = wtmp_pool.tile([128, 512], F32, tag="wtmp")
                nc.scalar.activation(
                    ws[:], win[:], mybir.ActivationFunctionType.Copy, scale=r_w
                )
                nc.vector.tensor_scalar(
                    ws[:], ws[:], 1.0, -1.0, mybir.AluOpType.min, mybir.AluOpType.max
                )
                wq = wq_pool.tile([128, 512], BF16)
                nc.vector.tensor_scalar(
                    wq[:], ws[:], MAGIC, MAGIC, mybir.AluOpType.add,
                    mybir.AluOpType.subtract,
                )
                for t in range(TT):
                    nc.tensor.matmul(
                        psums[t][:], xq[k][:, t * 128 : (t + 1) * 128], wq[:],
                        start=(k == 0), stop=(k == KT - 1),
                    )
            for t in range(TT):
                osb = ost_pool.tile([128, 512], F32)
                # out = psum * s_o + bias, one DVE op straight from PSUM
                nc.vector.scalar_tensor_tensor(
                    osb[:], psums[t][:], s_o, bias_rep[:, of : of + 512],
                    op0=mybir.AluOpType.mult, op1=mybir.AluOpType.add,
                )
                nc.sync.dma_start(
                    out[t * 128 : (t + 1) * 128, of : of + 512], osb[:]
                )

    nc.compile()
    return nc


def _prep_inputs(x, weight, bias):
    x2 = np.ascontiguousarray(x.reshape(TOKENS, IN_F).T)  # [IN_F, TOKENS]
    wT = np.ascontiguousarray(weight.T)  # [IN_F, OUT_F]
    in_maps = []
    for i in range(N_CORES):
        in_maps.append(
            {
                "xT": np.ascontiguousarray(x2[:, i * TPC : (i + 1) * TPC]),
                "wT": wT,
                "wS": np.ascontiguousarray(wT[:, i * OSL : (i + 1) * OSL]),
                "bias": bias,
            }
        )
    return in_maps


def _run(x, weight, bias, trace=False):
    if "nc" not in _cache:
        _cache["nc"] = _build()
    nc = _cache["nc"]
    in_maps = _prep_inputs(
        np.asarray(x, dtype=np.float32),
        np.asarray(weight, dtype=np.float32),
        np.asarray(bias, dtype=np.float32),
    )
    res = bass_utils.run_bass_kernel_spmd(
        nc, in_maps, list(range(N_CORES)), trace=trace
    )
    full = np.concatenate(
        [res.results[i]["out"] for i in range(N_CORES)], axis=0
    )
    return full.reshape(4, 2048, OUT_F), res


def kernel(x, weight, bias):
    out, _ = _run(x, weight, bias)
    return out


# revision 8
# speedup vs baseline: 1.1524x; 1.1524x over previous
# BitLinear 1.58 (ternary-weight linear with int8 activation quantization)
# on 8 Trainium2 NeuronCores via Bass/Tile.
#
# Reference computation (fp32):
#   w_scale = max(mean(|W|), 1e-5)           (global over the full weight)
#   W_q     = clip(round(W / w_scale), -1, 1)          (ternary)
#   gamma   = max(max(|x|), 1e-5)            (global over the full activation)
#   x_q     = clip(round(x * 128/gamma), -128, 127)
#   out     = (x_q @ W_q^T) * (gamma*w_scale/128) + bias
#
# Sharding: data-parallel over the 8192 tokens (1024 tokens/core), weight
# replicated. The global scales need cross-core reductions: each core
# computes a local absmax(x_shard) and a partial sum(|W|) over a distinct
# 1/8 slice of W, then two 4-byte AllGathers (one per stat, so the weight
# path and the activation path unblock independently); each core combines
# the gathered partials locally.
#
# The matmul contracts over in_features, which must live on the partition
# axis for both operands, so the host pre-transposes x and W once (layout
# prep, outside the device kernel). Quantized operands are fed to the PE in
# bf16 - exact here, because x_q in [-128,127] and W_q in {-1,0,1} are
# integers representable exactly in bf16, and PSUM accumulates in fp32
# (sums bounded by 4096*128 = 2^19 < 2^24, so accumulation is exact).
#
# Rounding: round-half-to-even (= jnp.round) done exactly in fp32 via the
# magic-constant trick (v + 1.5*2^23) - 1.5*2^23, fused into tensor_scalar
# ops. clip-then-round == round-then-clip at these bounds.
#
# Schedule notes: x-quantize is interleaved into the first of-column's
# k-loop so the DVE FIFO produces each wq[k] right when the PE needs it
# (a separate up-front x-quantize loop queues ~70us of DVE work ahead of
# the first weight tile and stalls the PE cold). Deep win prefetch hides
# the stats phase behind weight streaming.

import numpy as np
from contextlib import ExitStack

import concourse.bass as bass
import concourse.tile as tile
from concourse import bacc, mybir
from concourse import bass_utils

N_CORES = 8
IN_F = 4096
OUT_F = 4096
TOKENS = 8192  # 4 * 2048
TPC = TOKENS // N_CORES  # tokens per core = 1024
OSL = OUT_F // N_CORES  # per-core weight-stats slice = 512 out_features

KT = IN_F // 128  # 32 k-tiles
CT = OUT_F // 512  # 8 of-columns
TT = TPC // 128  # 8 token-tiles

MAGIC = 12582912.0  # 1.5 * 2**23: (v + MAGIC) - MAGIC == round-half-even(v)
EPS = 1e-5
F32 = mybir.dt.float32
BF16 = mybir.dt.bfloat16

_cache = {}


def _build(dbg=False):
    nc = bacc.Bacc("TRN2", target_bir_lowering=False, debug=False, num_devices=N_CORES)
    xT = nc.dram_tensor("xT", [IN_F, TPC], F32, kind="ExternalInput").ap()
    wT = nc.dram_tensor("wT", [IN_F, OUT_F], F32, kind="ExternalInput").ap()
    wS = nc.dram_tensor("wS", [IN_F, OSL], F32, kind="ExternalInput").ap()
    bias = nc.dram_tensor("bias", [OUT_F], F32, kind="ExternalInput").ap()
    out = nc.dram_tensor("out", [TPC, OUT_F], F32, kind="ExternalOutput").ap()
    if dbg:
        dbg_t = nc.dram_tensor("dbg", [16], F32, kind="ExternalOutput").ap()

    with tile.TileContext(nc) as tc, ExitStack() as ctx:
        ep = ctx.enter_context
        singles = ep(tc.tile_pool(name="singles", bufs=1))
        xin_pool = ep(tc.tile_pool(name="xin", bufs=4))
        xtmp_pool = ep(tc.tile_pool(name="xtmp", bufs=3))
        xq_pool = ep(tc.tile_pool(name="xq", bufs=KT))
        win_pool = ep(tc.tile_pool(name="win", bufs=12))
        wtmp_pool = ep(tc.tile_pool(name="wtmp", bufs=6))
        wq_pool = ep(tc.tile_pool(name="wq", bufs=6))
        ost_pool = ep(tc.tile_pool(name="ost", bufs=4))
        psum_pool = ep(tc.tile_pool(name="psum", bufs=8, space="PSUM"))
        dram = ep(tc.tile_pool(name="dram", bufs=1, space="DRAM"))

        ones_col = singles.tile([128, 1], F32)  # for partition-sum matmul
        nc.vector.memset(ones_col[:], 1.0)
        ones_row = singles.tile([1, 128], F32)  # for partition-broadcast matmul
        nc.vector.memset(ones_row[:], 1.0)

        # ---- stats: local absmax over x shard (x DMAs issued first) ----
        xm = singles.tile([128, KT], F32)
        for k in range(KT):
            xin = xin_pool.tile([128, TPC], F32)
            nc.sync.dma_start(xin[:], xT[k * 128 : (k + 1) * 128, :])
            nc.vector.tensor_reduce(
                xm[:, k : k + 1], xin[:], axis=mybir.AxisListType.X,
                op=mybir.AluOpType.max, apply_absolute_value=True,
            )
        xmax = singles.tile([128, 1], F32)
        nc.vector.tensor_reduce(
            xmax[:], xm[:], axis=mybir.AxisListType.X, op=mybir.AluOpType.max
        )
        xmaxT = singles.tile([1, 128], F32)
        nc.scalar.dma_start(xmaxT[:], xmax[:])  # cross-partition reshape
        gx = singles.tile([1, 1], F32)
        nc.vector.tensor_reduce(
            gx[:], xmaxT[:], axis=mybir.AxisListType.X, op=mybir.AluOpType.max
        )
        cc_xin = dram.tile([1], F32)
        cc_xout = dram.tile([N_CORES], F32)
        nc.scalar.dma_start(cc_xin[:], gx[:])
        nc.gpsimd.collective_compute(
            "AllGather", mybir.AluOpType.bypass,
            replica_groups=[list(range(N_CORES))],
            ins=[cc_xin.opt()], outs=[cc_xout.opt()],
        )

        # ---- stats: partial sum(|W|) over this core's slice ----
        wsum_ps = psum_pool.tile([128, OSL], F32, tag="ps")
        for k in range(KT):
            wsin = win_pool.tile([128, OSL], F32, tag="win")
            nc.sync.dma_start(wsin[:], wS[k * 128 : (k + 1) * 128, :])
            wabs = wtmp_pool.tile([128, OSL], F32, tag="wtmp")
            nc.scalar.activation(wabs[:], wsin[:], mybir.ActivationFunctionType.Abs)
            nc.tensor.matmul(
                wsum_ps[0:1, :], ones_col[:], wabs[:],
                start=(k == 0), stop=(k == KT - 1),
            )
        wsum = singles.tile([1, 1], F32)
        nc.vector.tensor_reduce(
            wsum[:], wsum_ps[0:1, :], axis=mybir.AxisListType.X,
            op=mybir.AluOpType.add,
        )
        cc_win = dram.tile([1], F32)
        cc_wout = dram.tile([N_CORES], F32)
        nc.scalar.dma_start(cc_win[:], wsum[:])
        nc.gpsimd.collective_compute(
            "AllGather", mybir.AluOpType.bypass,
            replica_groups=[list(range(N_CORES))],
            ins=[cc_win.opt()], outs=[cc_wout.opt()],
        )

        # ---- bias replicated across partitions (via K=1 matmul broadcast) ----
        bias_sb = singles.tile([1, OUT_F], F32)
        nc.sync.dma_start(bias_sb[:], bias[:])
        bias_rep = singles.tile([128, OUT_F], F32)
        for n in range(CT):
            bp = psum_pool.tile([128, 512], F32, tag="ps", name=f"biasps{n}")
            nc.tensor.matmul(
                bp[:], ones_row[:], bias_sb[0:1, n * 512 : (n + 1) * 512],
                start=True, stop=True,
            )
            nc.scalar.copy(bias_rep[:, n * 512 : (n + 1) * 512], bp[:])

        # ---- combine gathered stats; per-partition scalar math ----
        # w path first: wq production depends only on this
        g8w = singles.tile([1, N_CORES], F32)
        nc.scalar.dma_start(g8w[:], cc_wout[:])
        gsum = singles.tile([1, 1], F32)
        nc.vector.tensor_reduce(
            gsum[:], g8w[:], axis=mybir.AxisListType.X, op=mybir.AluOpType.add
        )
        wscale = singles.tile([1, 1], F32)
        nc.vector.tensor_scalar(
            wscale[:], gsum[:], 1.0 / (OUT_F * IN_F), EPS,
            mybir.AluOpType.mult, mybir.AluOpType.max,
        )
        ws_b = singles.tile([128, 1], F32)
        nc.gpsimd.partition_broadcast(ws_b[:], wscale[:])

        g8x = singles.tile([1, N_CORES], F32)
        nc.scalar.dma_start(g8x[:], cc_xout[:])
        gmax = singles.tile([1, 1], F32)
        nc.vector.tensor_reduce(
            gmax[:], g8x[:], axis=mybir.AxisListType.X, op=mybir.AluOpType.max
        )
        gamma = singles.tile([1, 1], F32)
        nc.vector.tensor_scalar(gamma[:], gmax[:], EPS, None, mybir.AluOpType.max)
        ga_b = singles.tile([128, 1], F32)
        nc.gpsimd.partition_broadcast(ga_b[:], gamma[:])

        def newton_recip(name, src):
            # correctly-rounded-ish 1/src: HW reciprocal + one Newton step
            r0 = singles.tile([128, 1], F32, tag=f"{name}r0")
            nc.vector.reciprocal(r0[:], src[:])
            t = singles.tile([128, 1], F32, tag=f"{name}t")
            nc.vector.tensor_tensor(t[:], src[:], r0[:], op=mybir.AluOpType.mult)
            u = singles.tile([128, 1], F32, tag=f"{name}u")
            nc.vector.tensor_scalar(
                u[:], t[:], -1.0, 2.0, mybir.AluOpType.mult, mybir.AluOpType.add
            )
            r1 = singles.tile([128, 1], F32, tag=f"{name}r1")
            nc.vector.tensor_tensor(r1[:], r0[:], u[:], op=mybir.AluOpType.mult)
            return r1

        rw_b = newton_recip("rw", ws_b)  # [128,1] = 1/w_scale
        r_w = rw_b[:, 0:1]

        rg_b = newton_recip("rg", ga_b)
        sx_b = singles.tile([128, 1], F32)
        nc.vector.tensor_scalar(
            sx_b[:], rg_b[:], 128.0, None, mybir.AluOpType.mult
        )
        s_x = sx_b[:, 0:1]

        so_b = singles.tile([128, 1], F32)
        nc.vector.tensor_tensor(so_b[:], ga_b[:], ws_b[:], op=mybir.AluOpType.mult)
        nc.vector.tensor_scalar(
            so_b[:], so_b[:], 2.0 ** -7, None, mybir.AluOpType.mult
        )
        s_o = so_b[:, 0:1]

        if dbg:
            dsb = singles.tile([1, 16], F32)
            nc.vector.memset(dsb[:], 0.0)
            nc.vector.tensor_copy(dsb[0:1, 0:1], gamma[:])
            nc.vector.tensor_copy(dsb[0:1, 1:2], wscale[:])
            nc.vector.tensor_copy(dsb[0:1, 2:3], sx_b[96:97, :])
            nc.vector.tensor_copy(dsb[0:1, 3:4], rw_b[96:97, :])
            nc.vector.tensor_copy(dsb[0:1, 4:5], so_b[96:97, :])
            nc.vector.tensor_copy(dsb[0:1, 8:9], gmax[:])
            nc.vector.tensor_copy(dsb[0:1, 9:10], gsum[:])
            nc.vector.tensor_copy(dsb[0:1, 10:11], gx[:])
            nc.vector.tensor_copy(dsb[0:1, 11:12], wsum[:])
            nc.sync.dma_start(dbg_t[:], dsb[:])

        # ---- main loop: stream W, ternarize, matmul, fused evict ----
        # x-quantize is interleaved into the c==0 k-loop (see header note)
        xq = [None] * KT

        def emit_xq(k):
            xin = xin_pool.tile([128, TPC], F32, tag="xin", name=f"xin_q{k}")
            nc.sync.dma_start(xin[:], xT[k * 128 : (k + 1) * 128, :])
            xs = xtmp_pool.tile([128, TPC], F32, tag="xtmp", name=f"xs{k}")
            nc.scalar.activation(
                xs[:], xin[:], mybir.ActivationFunctionType.Copy, scale=s_x
            )
            nc.vector.tensor_scalar(
                xs[:], xs[:], 127.0, -128.0, mybir.AluOpType.min, mybir.AluOpType.max
            )
            xq_k = xq_pool.tile([128, TPC], BF16, tag="xq", name=f"xq{k}")
            nc.vector.tensor_scalar(
                xq_k[:], xs[:], MAGIC, MAGIC, mybir.AluOpType.add,
                mybir.AluOpType.subtract,
            )
            xq[k] = xq_k

        for c in range(CT):
            of = c * 512
            psums = [
                psum_pool.tile([128, 512], F32, tag="ps", name=f"psum_c{c}_t{t}")
                for t in range(TT)
            ]
            for k in range(KT):
                if c == 0:
                    emit_xq(k)
                win = win_pool.tile([128, 512], F32, tag="win", name=f"win_c{c}_k{k}")
                nc.sync.dma_start(
                    win[:], wT[k * 128 : (k + 1) * 128, of : of + 512]
                )
                ws = wtmp_pool.tile([128, 512], F32, tag="wtmp", name=f"ws_c{c}_k{k}")
                nc.scalar.activation(
                    ws[:], win[:], mybir.ActivationFunctionType.Copy, scale=r_w
                )
                nc.vector.tensor_scalar(
                    ws[:], ws[:], 1.0, -1.0, mybir.AluOpType.min, mybir.AluOpType.max
                )
                wq = wq_pool.tile([128, 512], BF16, tag="wq", name=f"wq_c{c}_k{k}")
                nc.vector.tensor_scalar(
                    wq[:], ws[:], MAGIC, MAGIC, mybir.AluOpType.add,
                    mybir.AluOpType.subtract,
                )
                for t in range(TT):
                    nc.tensor.matmul(
                        psums[t][:], xq[k][:, t * 128 : (t + 1) * 128], wq[:],
                        start=(k == 0), stop=(k == KT - 1),
                    )
            for t in range(TT):
                osb = ost_pool.tile([128, 512], F32, tag="ost", name=f"osb_c{c}_t{t}")
                # out = psum * s_o + bias, one DVE op straight from PSUM
                nc.vector.scalar_tensor_tensor(
                    osb[:], psums[t][:], s_o, bias_rep[:, of : of + 512],
                    op0=mybir.AluOpType.mult, op1=mybir.AluOpType.add,
                )
                nc.sync.dma_start(
                    out[t * 128 : (t + 1) * 128, of : of + 512], osb[:]
                )

    nc.compile()
    return nc


def _prep_inputs(x, weight, bias):
    x2 = np.ascontiguousarray(x.reshape(TOKENS, IN_F).T)  # [IN_F, TOKENS]
    wT = np.ascontiguousarray(weight.T)  # [IN_F, OUT_F]
    in_maps = []
    for i in range(N_CORES):
        in_maps.append(
            {
                "xT": np.ascontiguousarray(x2[:, i * TPC : (i + 1) * TPC]),
                "wT": wT,
                "wS": np.ascontiguousarray(wT[:, i * OSL : (i + 1) * OSL]),
                "bias": bias,
            }
        )
    return in_maps


def _run(x, weight, bias, trace=False):
    if "nc" not in _cache:
        _cache["nc"] = _build()
    nc = _cache["nc"]
    in_maps = _prep_inputs(
        np.asarray(x, dtype=np.float32),
        np.asarray(weight, dtype=np.float32),
        np.asarray(bias, dtype=np.float32),
    )
    res = bass_utils.run_bass_kernel_spmd(
        nc, in_maps, list(range(N_CORES)), trace=trace
    )
    full = np.concatenate(
        [res.results[i]["out"] for i in range(N_CORES)], axis=0
    )
    return full.reshape(4, 2048, OUT_F), res


def kernel(x, weight, bias):
    out, _ = _run(x, weight, bias)
    return out


# revision 9
# speedup vs baseline: 1.1609x; 1.0074x over previous
# BitLinear 1.58 (ternary-weight linear with int8 activation quantization)
# on 8 Trainium2 NeuronCores via Bass/Tile.
#
# Reference computation (fp32):
#   w_scale = max(mean(|W|), 1e-5)           (global over the full weight)
#   W_q     = clip(round(W / w_scale), -1, 1)          (ternary)
#   gamma   = max(max(|x|), 1e-5)            (global over the full activation)
#   x_q     = clip(round(x * 128/gamma), -128, 127)
#   out     = (x_q @ W_q^T) * (gamma*w_scale/128) + bias
#
# Sharding: data-parallel over the 8192 tokens (1024 tokens/core), weight
# replicated. The global scales need cross-core reductions: each core
# computes a local absmax(x_shard) and a partial sum(|W|) over a distinct
# 1/8 slice of W, then two 4-byte AllGathers (one per stat, so the weight
# path and the activation path unblock independently); each core combines
# the gathered partials locally.
#
# The matmul contracts over in_features, which must live on the partition
# axis for both operands, so the host pre-transposes x and W once (layout
# prep, outside the device kernel). Quantized operands are fed to the PE in
# bf16 - exact here, because x_q in [-128,127] and W_q in {-1,0,1} are
# integers representable exactly in bf16, and PSUM accumulates in fp32
# (sums bounded by 4096*128 = 2^19 < 2^24, so accumulation is exact).
#
# Rounding: round-half-to-even (= jnp.round) done exactly in fp32 via the
# magic-constant trick (v + 1.5*2^23) - 1.5*2^23, fused into tensor_scalar
# ops. clip-then-round == round-then-clip at these bounds.
#
# Schedule notes: x-quantize is interleaved into the first of-column's
# k-loop so the DVE FIFO produces each wq[k] right when the PE needs it
# (a separate up-front x-quantize loop queues ~70us of DVE work ahead of
# the first weight tile and stalls the PE cold). Deep win prefetch hides
# the stats phase behind weight streaming.

import numpy as np
from contextlib import ExitStack

import concourse.bass as bass
import concourse.tile as tile
from concourse import bacc, mybir
from concourse import bass_utils

N_CORES = 8
IN_F = 4096
OUT_F = 4096
TOKENS = 8192  # 4 * 2048
TPC = TOKENS // N_CORES  # tokens per core = 1024
OSL = OUT_F // N_CORES  # per-core weight-stats slice = 512 out_features

KT = IN_F // 128  # 32 k-tiles
CT = OUT_F // 512  # 8 of-columns
TT = TPC // 128  # 8 token-tiles

MAGIC = 12582912.0  # 1.5 * 2**23: (v + MAGIC) - MAGIC == round-half-even(v)
EPS = 1e-5
F32 = mybir.dt.float32
BF16 = mybir.dt.bfloat16

_cache = {}


def _build(dbg=False):
    nc = bacc.Bacc("TRN2", target_bir_lowering=False, debug=False, num_devices=N_CORES)
    xT = nc.dram_tensor("xT", [IN_F, TPC], F32, kind="ExternalInput").ap()
    wT = nc.dram_tensor("wT", [IN_F, OUT_F], F32, kind="ExternalInput").ap()
    wS = nc.dram_tensor("wS", [IN_F, OSL], F32, kind="ExternalInput").ap()
    bias = nc.dram_tensor("bias", [OUT_F], F32, kind="ExternalInput").ap()
    out = nc.dram_tensor("out", [TPC, OUT_F], F32, kind="ExternalOutput").ap()
    if dbg:
        dbg_t = nc.dram_tensor("dbg", [16], F32, kind="ExternalOutput").ap()

    with tile.TileContext(nc) as tc, ExitStack() as ctx:
        ep = ctx.enter_context
        singles = ep(tc.tile_pool(name="singles", bufs=1))
        xin_pool = ep(tc.tile_pool(name="xin", bufs=6))
        xq_pool = ep(tc.tile_pool(name="xq", bufs=KT))
        win_pool = ep(tc.tile_pool(name="win", bufs=24))
        wq_pool = ep(tc.tile_pool(name="wq", bufs=8))
        ost_pool = ep(tc.tile_pool(name="ost", bufs=4))
        psum_pool = ep(tc.tile_pool(name="psum", bufs=8, space="PSUM"))
        dram = ep(tc.tile_pool(name="dram", bufs=1, space="DRAM"))

        ones_col = singles.tile([128, 1], F32)  # for partition-sum matmul
        nc.vector.memset(ones_col[:], 1.0)
        ones_row = singles.tile([1, 128], F32)  # for partition-broadcast matmul
        nc.vector.memset(ones_row[:], 1.0)

        # ---- stats: local absmax over x shard (x DMAs issued first) ----
        xm = singles.tile([128, KT], F32)
        for k in range(KT):
            xin = xin_pool.tile([128, TPC], F32)
            nc.sync.dma_start(xin[:], xT[k * 128 : (k + 1) * 128, :])
            nc.vector.tensor_reduce(
                xm[:, k : k + 1], xin[:], axis=mybir.AxisListType.X,
                op=mybir.AluOpType.max, apply_absolute_value=True,
            )
        xmax = singles.tile([128, 1], F32)
        nc.vector.tensor_reduce(
            xmax[:], xm[:], axis=mybir.AxisListType.X, op=mybir.AluOpType.max
        )
        xmaxT = singles.tile([1, 128], F32)
        nc.scalar.dma_start(xmaxT[:], xmax[:])  # cross-partition reshape
        gx = singles.tile([1, 1], F32)
        nc.vector.tensor_reduce(
            gx[:], xmaxT[:], axis=mybir.AxisListType.X, op=mybir.AluOpType.max
        )
        cc_xin = dram.tile([1], F32)
        cc_xout = dram.tile([N_CORES], F32)
        nc.scalar.dma_start(cc_xin[:], gx[:])
        nc.gpsimd.collective_compute(
            "AllGather", mybir.AluOpType.bypass,
            replica_groups=[list(range(N_CORES))],
            ins=[cc_xin.opt()], outs=[cc_xout.opt()],
        )

        # ---- stats: partial sum(|W|) over this core's slice ----
        wsum_ps = psum_pool.tile([128, OSL], F32, tag="ps")
        for k in range(KT):
            wsin = win_pool.tile([128, OSL], F32, tag="win")
            nc.sync.dma_start(wsin[:], wS[k * 128 : (k + 1) * 128, :])
            nc.scalar.activation(wsin[:], wsin[:], mybir.ActivationFunctionType.Abs)
            nc.tensor.matmul(
                wsum_ps[0:1, :], ones_col[:], wsin[:],
                start=(k == 0), stop=(k == KT - 1),
            )
        wsum = singles.tile([1, 1], F32)
        nc.vector.tensor_reduce(
            wsum[:], wsum_ps[0:1, :], axis=mybir.AxisListType.X,
            op=mybir.AluOpType.add,
        )
        cc_win = dram.tile([1], F32)
        cc_wout = dram.tile([N_CORES], F32)
        nc.scalar.dma_start(cc_win[:], wsum[:])
        nc.gpsimd.collective_compute(
            "AllGather", mybir.AluOpType.bypass,
            replica_groups=[list(range(N_CORES))],
            ins=[cc_win.opt()], outs=[cc_wout.opt()],
        )

        # ---- bias replicated across partitions (via K=1 matmul broadcast) ----
        bias_sb = singles.tile([1, OUT_F], F32)
        nc.sync.dma_start(bias_sb[:], bias[:])
        bias_rep = singles.tile([128, OUT_F], F32)
        for n in range(CT):
            bp = psum_pool.tile([128, 512], F32, tag="ps", name=f"biasps{n}")
            nc.tensor.matmul(
                bp[:], ones_row[:], bias_sb[0:1, n * 512 : (n + 1) * 512],
                start=True, stop=True,
            )
            nc.scalar.copy(bias_rep[:, n * 512 : (n + 1) * 512], bp[:])

        # ---- combine gathered stats; per-partition scalar math ----
        # w path first: wq production depends only on this
        g8w = singles.tile([1, N_CORES], F32)
        nc.scalar.dma_start(g8w[:], cc_wout[:])
        gsum = singles.tile([1, 1], F32)
        nc.vector.tensor_reduce(
            gsum[:], g8w[:], axis=mybir.AxisListType.X, op=mybir.AluOpType.add
        )
        wscale = singles.tile([1, 1], F32)
        nc.vector.tensor_scalar(
            wscale[:], gsum[:], 1.0 / (OUT_F * IN_F), EPS,
            mybir.AluOpType.mult, mybir.AluOpType.max,
        )
        ws_b = singles.tile([128, 1], F32)
        nc.gpsimd.partition_broadcast(ws_b[:], wscale[:])

        g8x = singles.tile([1, N_CORES], F32)
        nc.scalar.dma_start(g8x[:], cc_xout[:])
        gmax = singles.tile([1, 1], F32)
        nc.vector.tensor_reduce(
            gmax[:], g8x[:], axis=mybir.AxisListType.X, op=mybir.AluOpType.max
        )
        gamma = singles.tile([1, 1], F32)
        nc.vector.tensor_scalar(gamma[:], gmax[:], EPS, None, mybir.AluOpType.max)
        ga_b = singles.tile([128, 1], F32)
        nc.gpsimd.partition_broadcast(ga_b[:], gamma[:])

        def newton_recip(name, src):
            # correctly-rounded-ish 1/src: HW reciprocal + one Newton step
            r0 = singles.tile([128, 1], F32, tag=f"{name}r0")
            nc.vector.reciprocal(r0[:], src[:])
            t = singles.tile([128, 1], F32, tag=f"{name}t")
            nc.vector.tensor_tensor(t[:], src[:], r0[:], op=mybir.AluOpType.mult)
            u = singles.tile([128, 1], F32, tag=f"{name}u")
            nc.vector.tensor_scalar(
                u[:], t[:], -1.0, 2.0, mybir.AluOpType.mult, mybir.AluOpType.add
            )
            r1 = singles.tile([128, 1], F32, tag=f"{name}r1")
            nc.vector.tensor_tensor(r1[:], r0[:], u[:], op=mybir.AluOpType.mult)
            return r1

        rw_b = newton_recip("rw", ws_b)  # [128,1] = 1/w_scale
        r_w = rw_b[:, 0:1]

        rg_b = newton_recip("rg", ga_b)
        sx_b = singles.tile([128, 1], F32)
        nc.vector.tensor_scalar(
            sx_b[:], rg_b[:], 128.0, None, mybir.AluOpType.mult
        )
        s_x = sx_b[:, 0:1]

        so_b = singles.tile([128, 1], F32)
        nc.vector.tensor_tensor(so_b[:], ga_b[:], ws_b[:], op=mybir.AluOpType.mult)
        nc.vector.tensor_scalar(
            so_b[:], so_b[:], 2.0 ** -7, None, mybir.AluOpType.mult
        )
        s_o = so_b[:, 0:1]

        if dbg:
            dsb = singles.tile([1, 16], F32)
            nc.vector.memset(dsb[:], 0.0)
            nc.vector.tensor_copy(dsb[0:1, 0:1], gamma[:])
            nc.vector.tensor_copy(dsb[0:1, 1:2], wscale[:])
            nc.vector.tensor_copy(dsb[0:1, 2:3], sx_b[96:97, :])
            nc.vector.tensor_copy(dsb[0:1, 3:4], rw_b[96:97, :])
            nc.vector.tensor_copy(dsb[0:1, 4:5], so_b[96:97, :])
            nc.vector.tensor_copy(dsb[0:1, 8:9], gmax[:])
            nc.vector.tensor_copy(dsb[0:1, 9:10], gsum[:])
            nc.vector.tensor_copy(dsb[0:1, 10:11], gx[:])
            nc.vector.tensor_copy(dsb[0:1, 11:12], wsum[:])
            nc.sync.dma_start(dbg_t[:], dsb[:])

        # ---- main loop: stream W, ternarize, matmul, fused evict ----
        # x-quantize is interleaved into the c==0 k-loop (see header note)
        xq = [None] * KT

        def emit_xq(k):
            xin = xin_pool.tile([128, TPC], F32, tag="xin", name=f"xin_q{k}")
            nc.scalar.dma_start(xin[:], xT[k * 128 : (k + 1) * 128, :])
            nc.scalar.activation(
                xin[:], xin[:], mybir.ActivationFunctionType.Copy, scale=s_x
            )
            nc.vector.tensor_scalar(
                xin[:], xin[:], 127.0, -128.0, mybir.AluOpType.min,
                mybir.AluOpType.max,
            )
            xq_k = xq_pool.tile([128, TPC], BF16, tag="xq", name=f"xq{k}")
            nc.vector.tensor_scalar(
                xq_k[:], xin[:], MAGIC, MAGIC, mybir.AluOpType.add,
                mybir.AluOpType.subtract,
            )
            xq[k] = xq_k

        for c in range(CT):
            of = c * 512
            psums = [
                psum_pool.tile([128, 512], F32, tag="ps", name=f"psum_c{c}_t{t}")
                for t in range(TT)
            ]
            for k in range(KT):
                if c == 0:
                    emit_xq(k)
                win = win_pool.tile([128, 512], F32, tag="win", name=f"win_c{c}_k{k}")
                nc.sync.dma_start(
                    win[:], wT[k * 128 : (k + 1) * 128, of : of + 512]
                )
                nc.scalar.activation(
                    win[:], win[:], mybir.ActivationFunctionType.Copy, scale=r_w
                )
                nc.vector.tensor_scalar(
                    win[:], win[:], 1.0, -1.0, mybir.AluOpType.min,
                    mybir.AluOpType.max,
                )
                wq = wq_pool.tile([128, 512], BF16, tag="wq", name=f"wq_c{c}_k{k}")
                nc.vector.tensor_scalar(
                    wq[:], win[:], MAGIC, MAGIC, mybir.AluOpType.add,
                    mybir.AluOpType.subtract,
                )
                for t in range(TT):
                    nc.tensor.matmul(
                        psums[t][:], xq[k][:, t * 128 : (t + 1) * 128], wq[:],
                        start=(k == 0), stop=(k == KT - 1),
                    )
            for t in range(TT):
                osb = ost_pool.tile([128, 512], F32, tag="ost", name=f"osb_c{c}_t{t}")
                # out = psum * s_o + bias, one DVE op straight from PSUM
                nc.vector.scalar_tensor_tensor(
                    osb[:], psums[t][:], s_o, bias_rep[:, of : of + 512],
                    op0=mybir.AluOpType.mult, op1=mybir.AluOpType.add,
                )
                nc.sync.dma_start(
                    out[t * 128 : (t + 1) * 128, of : of + 512], osb[:]
                )

    nc.compile()
    return nc


def _prep_inputs(x, weight, bias):
    x2 = np.ascontiguousarray(x.reshape(TOKENS, IN_F).T)  # [IN_F, TOKENS]
    wT = np.ascontiguousarray(weight.T)  # [IN_F, OUT_F]
    in_maps = []
    for i in range(N_CORES):
        in_maps.append(
            {
                "xT": np.ascontiguousarray(x2[:, i * TPC : (i + 1) * TPC]),
                "wT": wT,
                "wS": np.ascontiguousarray(wT[:, i * OSL : (i + 1) * OSL]),
                "bias": bias,
            }
        )
    return in_maps


def _run(x, weight, bias, trace=False):
    if "nc" not in _cache:
        _cache["nc"] = _build()
    nc = _cache["nc"]
    in_maps = _prep_inputs(
        np.asarray(x, dtype=np.float32),
        np.asarray(weight, dtype=np.float32),
        np.asarray(bias, dtype=np.float32),
    )
    res = bass_utils.run_bass_kernel_spmd(
        nc, in_maps, list(range(N_CORES)), trace=trace
    )
    full = np.concatenate(
        [res.results[i]["out"] for i in range(N_CORES)], axis=0
    )
    return full.reshape(4, 2048, OUT_F), res


def kernel(x, weight, bias):
    out, _ = _run(x, weight, bias)
    return out
